# revision 9
# baseline (speedup 1.0000x reference)
"""Trainium2 Bass kernel for a 2-layer GCN encoder (GCNConv -> ReLU -> {GCNConv mu, GCNConv logstd}).

Strategy (8 NeuronCores, SPMD):
  - Math: propagate(M) = D^-1/2 (A+I) D^-1/2 M  ==  d * ((A+I) @ (d * M)) with d = deg^-1/2,
    so per-edge norm weights disappear: scale rows by d before and after message passing.
  - Layers 2 and 3 share the propagate: fuse W_mu/W_logstd into one [128,128] matmul + one
    message-passing pass over 128 features, split on the host afterwards.
  - Sharding: nodes are partitioned across the 8 cores (dst-sharding). Each core owns
    N/8 = 6250 output rows and processes the ~E/8 edges pointing into them.
  - Layer-1 linear (x @ W1.T) is replicated on every core (reads all of x, ~51 MB bf16)
    which avoids an all-gather of the layer-1 message table. Layer-2 tables require one
    AllGather of the per-core [6250,128] shards.
  - Message passing: dma_gather (HW gather, int16 indices) pulls source-node rows from the
    HBM-resident message table; a one-hot selection matrix (built on DVE via is_equal vs an
    iota row) turns the segment-sum into PE matmuls accumulated in PSUM per 128-dst-node tile.
  - int16 gather indices only address <=32767 rows, so node tables are split in two halves
    and each core's edge list is bucketed by source half (then by dst tile, padded to
    multiples of 128 with dummy edges whose one-hot column is out of range).
  - All cores run the same program (SPMD): per-(half,tile) group sizes are padded to the
    max over cores.

kernel(**inputs) takes the full-size inputs and returns (mu, logstd) as float32 numpy arrays.
"""
import sys

sys.path.insert(0, "/opt/trn_rl_repo")

import numpy as np
import ml_dtypes

import concourse.bass as bass
import concourse.bacc as bacc
import concourse.mybir as mybir
import concourse.tile as tile
from concourse.bass_utils import run_bass_kernel_spmd

BF16 = ml_dtypes.bfloat16

# ---------------- configuration ----------------
FULL_CFG = dict(
    n=50000,        # nodes
    fin=512,        # input features
    hid=128,        # hidden features
    out2=128,       # fused mu+logstd features
    n_cores=8,
    npad=51200,     # padded node count (multiple of 512)
    hsplit=25600,   # node-table half split (multiple of 512, both halves <= 32767)
    panel=2048,     # phase-A x_t panel width
    blk=512,        # phase-A block width
    g_edges=8192,   # gather super-chunk (edges per dma_gather)
    single_packet=False,  # >1024 idxs per gather needs multi-packet drain
    table_bf16=True,
    mm1_bf16=True,
    reps=1,         # kernel body repetitions (for timing)
)


def _ceil(a, b):
    return -(-a // b)


def preprocess(cfg, x, edge_index, W1, b1, W_mu, b_mu, W_logstd, b_logstd):
    """Host-side: degrees, edge bucketing/padding, operand staging. Returns
    (meta, in_maps). Pure index/layout work plus parameter reformatting."""
    N, C = cfg["n"], cfg["n_cores"]
    NPC = N // C
    T = _ceil(NPC, 128)
    HS = cfg["hsplit"]
    G = cfg["g_edges"]
    t_dt = BF16 if cfg["table_bf16"] else np.float32
    m_dt = BF16 if cfg["mm1_bf16"] else np.float32

    x = np.asarray(x, np.float32)
    ei = np.asarray(edge_index).astype(np.int64)
    W1 = np.asarray(W1, np.float32)
    b1 = np.asarray(b1, np.float32)
    Wcat = np.concatenate([np.asarray(W_mu, np.float32), np.asarray(W_logstd, np.float32)], axis=0)
    bcat = np.concatenate([np.asarray(b_mu, np.float32), np.asarray(b_logstd, np.float32)], axis=0)

    src = np.concatenate([ei[0], np.arange(N, dtype=np.int64)])
    dst = np.concatenate([ei[1], np.arange(N, dtype=np.int64)])
    deg = np.bincount(dst, minlength=N).astype(np.float32)
    dvec = (1.0 / np.sqrt(deg)).astype(np.float32)

    core = dst // NPC
    tloc = (dst % NPC) // 128
    half = (src >= HS).astype(np.int64)
    key = (core * T + tloc) * 2 + half
    order = np.argsort(key, kind="stable")
    ks, ss, ds = key[order], src[order], dst[order]
    counts = np.bincount(ks, minlength=C * T * 2).reshape(C, T, 2)
    gpad = ((counts.max(axis=0) + 127) // 128) * 128  # [T, 2] padded group sizes
    Lh = gpad.sum(axis=0)  # per-half padded edge totals (same for all cores)
    cpt = (gpad // 128)    # chunks per (tile, half)
    offs = np.concatenate([[0], np.cumsum(counts.reshape(-1))])

    # phase-A staging
    NPAD = cfg["npad"]
    xt = np.zeros((cfg["fin"], NPAD), m_dt)
    xt[:, :N] = x.T
    w1t = np.ascontiguousarray(W1.T).astype(m_dt)          # [fin, hid]
    wcatt = np.ascontiguousarray(Wcat.T).astype(t_dt)      # [hid, out2]
    d_all = np.ones(NPAD, np.float32)
    d_all[:N] = dvec
    d_all_col = np.ascontiguousarray(d_all.reshape(-1, 128).T)  # [128, NPAD/128]
    iota_arr = np.tile(np.arange(128), (128, 1)).astype(t_dt)
    ident = np.eye(128, dtype=t_dt)
    identf = np.eye(128, dtype=np.float32)

    K_tot = int(Lh.sum() // 128)
    in_maps = []
    for c in range(C):
        # edge order: (tile, half) groups; gather idx streams are per half in
        # tile order; dstloc columns are (tile: half0-chunks then half1-chunks).
        bufs_ = [np.zeros(int(Lh[0]), np.int16), np.zeros(int(Lh[1]), np.int16)]
        ph = [0, 0]
        dstloc_all = np.full(int(Lh.sum()), 200, np.int32)
        pos = 0
        for t in range(T):
            for h in (0, 1):
                g = int(counts[c, t, h])
                o = int(offs[(c * T + t) * 2 + h])
                sl = slice(o, o + g)
                bufs_[h][ph[h]:ph[h] + g] = (ss[sl] - h * HS).astype(np.int16)
                dstloc_all[pos:pos + g] = (ds[sl] % NPC) - t * 128
                ph[h] += int(gpad[t, h])
                pos += int(gpad[t, h])
        idx_h = [np.tile(b.reshape(-1, 16).T, (8, 1)).copy() if b.size
                 else np.zeros((128, 1), np.int16) for b in bufs_]
        dstloc_arr = np.ascontiguousarray(dstloc_all.reshape(-1, 128).T).astype(t_dt)

        d_own = dvec[c * NPC:(c + 1) * NPC]
        d_own_pad = np.ones(T * 128, np.float32)
        d_own_pad[:NPC] = d_own
        d_rep = np.tile(d_own_pad, (128, 1)).astype(np.float32)          # [128, T*128]
        d_own_col = np.ascontiguousarray(d_own_pad.reshape(-1, 128).T)   # [128, T]

        in_maps.append({
            "xt": xt, "w1t": w1t, "wcatt": wcatt,
            "b1c": b1.reshape(-1, 1).copy(), "bcatc": bcat.reshape(-1, 1).copy(),
            "dallc": d_all_col, "drep": d_rep, "downc": d_own_col,
            "iota": iota_arr, "ident": ident, "identf": identf,
            "idx0": idx_h[0], "idx1": idx_h[1], "dstloc": dstloc_arr,
        })

    meta = dict(cpt=cpt.tolist(), Lh=[int(Lh[0]), int(Lh[1])], K_tot=K_tot)
    return meta, in_maps


def build_program(cfg, meta):
    N, C = cfg["n"], cfg["n_cores"]
    NPC = N // C
    T = _ceil(NPC, 128)
    HS, NPAD = cfg["hsplit"], cfg["npad"]
    FIN, HID, O2 = cfg["fin"], cfg["hid"], cfg["out2"]
    PANEL, BLK, G = cfg["panel"], cfg["blk"], cfg["g_edges"]
    KC = FIN // 128
    SPC = G // 128  # chunk slots per gather super-chunk
    cpt, Lh = meta["cpt"], meta["Lh"]
    K_tot = meta["K_tot"]
    dt_tab = mybir.dt.bfloat16 if cfg["table_bf16"] else mybir.dt.float32
    dt_mm = mybir.dt.bfloat16 if cfg["mm1_bf16"] else mybir.dt.float32
    f32 = mybir.dt.float32
    AF = mybir.ActivationFunctionType
    OP = mybir.AluOpType

    nc = bacc.Bacc("TRN2", target_bir_lowering=False, debug=False, num_devices=C,
                   num_swdge_queues=cfg.get("swdge_queues", 1))

    xt_d = nc.dram_tensor("xt", [FIN, NPAD], dt_mm, kind="ExternalInput")
    w1t_d = nc.dram_tensor("w1t", [FIN, HID], dt_mm, kind="ExternalInput")
    wcatt_d = nc.dram_tensor("wcatt", [HID, O2], dt_tab, kind="ExternalInput")
    b1c_d = nc.dram_tensor("b1c", [HID, 1], f32, kind="ExternalInput")
    bcatc_d = nc.dram_tensor("bcatc", [O2, 1], f32, kind="ExternalInput")
    dallc_d = nc.dram_tensor("dallc", [128, NPAD // 128], f32, kind="ExternalInput")
    drep_d = nc.dram_tensor("drep", [128, T * 128], f32, kind="ExternalInput")
    downc_d = nc.dram_tensor("downc", [128, T], f32, kind="ExternalInput")
    iota_d = nc.dram_tensor("iota", [128, 128], dt_tab, kind="ExternalInput")
    ident_d = nc.dram_tensor("ident", [128, 128], dt_tab, kind="ExternalInput")
    identf_d = nc.dram_tensor("identf", [128, 128], f32, kind="ExternalInput")
    idx_d = [nc.dram_tensor(f"idx{h}", [128, max(Lh[h] // 16, 1)], mybir.dt.int16,
                            kind="ExternalInput") for h in (0, 1)]
    dstloc_d = nc.dram_tensor("dstloc", [128, max(K_tot, 1)], dt_tab, kind="ExternalInput")

    g1_d = [nc.dram_tensor("g1a", [HS, HID], dt_tab),
            nc.dram_tensor("g1b", [NPAD - HS, HID], dt_tab)]
    g2s_d = nc.dram_tensor("g2s", [NPC, HID], dt_tab)
    g2f_d = nc.dram_tensor("g2f", [N, HID], dt_tab, addr_space="Shared")
    # dma_gather cannot source from a Shared-address tensor (device crash);
    # bounce the collective output into plain DRAM halves.
    g2c_d = [nc.dram_tensor("g2ca", [HS, HID], dt_tab),
             nc.dram_tensor("g2cb", [N - HS, HID], dt_tab)]
    outt_d = nc.dram_tensor("outt", [O2, T * 128], f32, kind="ExternalOutput")

    with tile.TileContext(nc, trace_sim=bool(cfg.get("trace_sim"))) as tc:
        with tc.tile_pool(name="const", bufs=1) as const_p:
            w1t_sb = []
            for kc in range(KC):
                w = const_p.tile([128, HID], dt_mm, tag=f"w1t{kc}")
                nc.sync.dma_start(w[:], w1t_d[kc * 128:(kc + 1) * 128, :])
                w1t_sb.append(w)
            wcatt_sb = const_p.tile([HID, O2], dt_tab, tag="wcatt")
            nc.sync.dma_start(wcatt_sb[:], wcatt_d[:])
            b1_sb = const_p.tile([HID, 1], f32, tag="b1")
            nc.sync.dma_start(b1_sb[:], b1c_d[:])
            bcat_sb = const_p.tile([O2, 1], f32, tag="bcat")
            nc.sync.dma_start(bcat_sb[:], bcatc_d[:])
            dallc_sb = const_p.tile([128, NPAD // 128], f32, tag="dallc")
            nc.sync.dma_start(dallc_sb[:], dallc_d[:])
            drep_sb = const_p.tile([128, T * 128], f32, tag="drep")
            nc.sync.dma_start(drep_sb[:], drep_d[:])
            downc_sb = const_p.tile([128, T], f32, tag="downc")
            nc.sync.dma_start(downc_sb[:], downc_d[:])
            iota_sb = const_p.tile([128, 128], dt_tab, tag="iota")
            nc.sync.dma_start(iota_sb[:], iota_d[:])
            ident_sb = const_p.tile([128, 128], dt_tab, tag="ident")
            nc.sync.dma_start(ident_sb[:], ident_d[:])
            ident_f32_sb = const_p.tile([128, 128], f32, tag="identf")
            nc.sync.dma_start(ident_f32_sb[:], identf_d[:])
            idx_sb = []
            for h in (0, 1):
                t_ = const_p.tile([128, max(Lh[h] // 16, 1)], mybir.dt.int16, tag=f"idx{h}")
                nc.sync.dma_start(t_[:], idx_d[h][:])
                idx_sb.append(t_)
            dstloc_sb = const_p.tile([128, max(K_tot, 1)], dt_tab, tag="dstloc")
            nc.sync.dma_start(dstloc_sb[:], dstloc_d[:])

            for _rep in range(cfg.get("reps", 1)):
                with nc.named_scope("body"):
                    _emit_body(nc, tc, cfg, meta, locals())

    nc.compile()
    return nc


def _emit_body(nc, tc, cfg, meta, env):
    """One full forward pass. `env` carries the SBUF constants + DRAM handles."""
    N, C = cfg["n"], cfg["n_cores"]
    NPC = N // C
    T = _ceil(NPC, 128)
    HS, NPAD = cfg["hsplit"], cfg["npad"]
    FIN, HID, O2 = cfg["fin"], cfg["hid"], cfg["out2"]
    PANEL, BLK, G = cfg["panel"], cfg["blk"], cfg["g_edges"]
    KC = FIN // 128
    SPC = G // 128
    cpt, Lh = meta["cpt"], meta["Lh"]
    dt_tab = mybir.dt.bfloat16 if cfg["table_bf16"] else mybir.dt.float32
    dt_mm = mybir.dt.bfloat16 if cfg["mm1_bf16"] else mybir.dt.float32
    f32 = mybir.dt.float32
    AF = mybir.ActivationFunctionType
    OP = mybir.AluOpType

    xt_d, dstloc_sb, idx_sb = env["xt_d"], env["dstloc_sb"], env["idx_sb"]
    g1_d, g2s_d, g2f_d, outt_d = env["g1_d"], env["g2s_d"], env["g2f_d"], env["outt_d"]
    g2c_d = env["g2c_d"]
    w1t_sb, wcatt_sb = env["w1t_sb"], env["wcatt_sb"]
    b1_sb, bcat_sb = env["b1_sb"], env["bcat_sb"]
    dallc_sb, drep_sb, downc_sb = env["dallc_sb"], env["drep_sb"], env["downc_sb"]
    iota_sb, ident_sb = env["iota_sb"], env["ident_sb"]
    ident_f32_sb = env["ident_f32_sb"]

    # ---------------- phase A: g1 = d * (x @ W1.T), replicated, written to HBM tables
    sc = nc.named_scope("phaseA")
    sc.__enter__()
    with tc.tile_pool(name="pa_x", bufs=2) as xt_p, \
         tc.tile_pool(name="pa_t", bufs=3) as t1_p, \
         tc.tile_pool(name="pa_w", bufs=3) as wst_p, \
         tc.tile_pool(name="pa_ps", bufs=2, space="PSUM") as pa, \
         tc.tile_pool(name="pa_ps2", bufs=2, space="PSUM") as pb:
        for pan in range(NPAD // PANEL):
            xts = []
            for kc in range(KC):
                xk = xt_p.tile([128, PANEL], dt_mm, tag=f"xt{kc}")
                nc.sync.dma_start(xk[:], xt_d[kc * 128:(kc + 1) * 128,
                                              pan * PANEL:(pan + 1) * PANEL])
                xts.append(xk)
            for bi in range(PANEL // BLK):
                blk = pan * (PANEL // BLK) + bi
                ps_a = pa.tile([128, BLK], f32, space="PSUM", tag="psa")
                for kc in range(KC):
                    nc.tensor.matmul(ps_a[:], lhsT=w1t_sb[kc][:],
                                     rhs=xts[kc][:, bi * BLK:(bi + 1) * BLK],
                                     start=(kc == 0), stop=(kc == KC - 1))
                t1t = t1_p.tile([128, BLK], f32, tag="t1t")
                nc.scalar.copy(t1t[:], ps_a[:])
                wst = wst_p.tile([128, BLK // 128, HID], dt_tab, tag="wst")
                sb = BLK // 128
                ps_b = pb.tile([128, sb, 128], f32, space="PSUM", tag="psb")
                for s in range(sb):
                    nc.tensor.transpose(ps_b[:, s, :], t1t[:, s * 128:(s + 1) * 128],
                                        ident_f32_sb[:])
                # scale all sub-blocks by their per-node d in one DVE op:
                # wst[p, s, f] = ps_b[p, s, f] * d[(nb0+s)*128 + p]
                nb0 = blk * sb
                dsl = dallc_sb[:, nb0:nb0 + sb]
                in1 = bass.AP(dsl.tensor, dsl.offset,
                              [dsl.ap[0], [dsl.ap[1][0], sb], [0, 128]])
                nc.vector.tensor_tensor(out=wst[:], in0=ps_b[:], in1=in1, op=OP.mult)
                r0 = blk * BLK
                tab, roff = (g1_d[0], r0) if r0 < HS else (g1_d[1], r0 - HS)
                nc.sync.dma_start(
                    tab[roff:roff + BLK, :].rearrange("(s p) f -> p s f", p=128), wst[:])

    sc.__exit__(None, None, None)
    if cfg.get("stop_after") == "A":
        return

    # ---------------- message passing (used for both layers)
    def propagate(tables, finalize, gathers_only=False):
        with tc.tile_pool(name="mp_g", bufs=2) as gath_p, \
             tc.tile_pool(name="mp_oh", bufs=3) as oh_p, \
             tc.tile_pool(name="mp_ps", bufs=4, space="PSUM") as psp:
            gh = {}
            nq = cfg.get("swdge_queues", 1)
            qctr = 0
            for h in (0, 1):
                for i in range(_ceil(Lh[h], G)):
                    n_i = min(G, Lh[h] - i * G)
                    gt = gath_p.tile([128, SPC, HID], dt_tab, tag=f"g{h}", name="gt")
                    nc.gpsimd.dma_gather(
                        out_ap=gt[:, :n_i // 128, :],
                        in_ap=tables[h],
                        idxs_ap=idx_sb[h][:, i * (G // 16):i * (G // 16) + n_i // 16],
                        num_idxs=n_i,
                        num_idxs_reg=n_i,
                        elem_size=HID,
                        single_packet=cfg.get("single_packet", True),
                        queue_num=qctr % nq,
                    )
                    qctr += 1
                    gh[(h, i)] = gt
            if gathers_only:
                return
            # Two passes (half 0 then half 1) so half-0 chunk consumption only
            # depends on the half-0 table; partials accumulate in SBUF.
            kh = [0, 0]
            acc = {}
            for h in (0, 1):
                for t in range(T):
                    # dstloc columns for tile t: [kg0(t) .. ] half0 then half1
                    kg = sum(cpt[tt][0] + cpt[tt][1] for tt in range(t))
                    if h == 1:
                        kg += cpt[t][0]
                    nch = cpt[t][h]
                    if nch == 0:
                        if h == 1:
                            a = acc.get(t)
                            if a is None:
                                a = env["acc_p"].tile([128, 128], f32,
                                                      tag=f"acc{t}", name="acc_t")
                                nc.vector.memset(a[:], 0.0)
                            finalize(t, a)
                        continue
                    oh = oh_p.tile([128, nch, 128], dt_tab, tag="oh", name="oh")
                    dsl = dstloc_sb[:, kg:kg + nch]
                    in0 = bass.AP(dsl.tensor, dsl.offset,
                                  [dsl.ap[0], [dsl.ap[1][0], nch], [0, 128]])
                    io = iota_sb[:]
                    in1 = bass.AP(io.tensor, io.offset, [io.ap[0], [0, nch], io.ap[1]])
                    nc.vector.tensor_tensor(out=oh[:], in0=in0, in1=in1, op=OP.is_equal)
                    ps_t = psp.tile([128, 128], f32, space="PSUM", tag="ps", name="ps_t")
                    for j in range(nch):
                        gt = gh[(h, kh[h] // SPC)]
                        nc.tensor.matmul(ps_t[:], lhsT=gt[:, kh[h] % SPC, :],
                                         rhs=oh[:, j, :],
                                         start=(j == 0), stop=(j == nch - 1))
                        kh[h] += 1
                    if h == 0:
                        a = env["acc_p"].tile([128, 128], f32, tag=f"acc{t}", name="acc_t")
                        nc.scalar.copy(a[:], ps_t[:])
                        acc[t] = a
                    else:
                        a = acc.get(t)
                        if a is not None:
                            nc.vector.tensor_tensor(out=a[:], in0=a[:], in1=ps_t[:],
                                                    op=OP.add)
                            finalize(t, a)
                        else:
                            finalize(t, ps_t)

    with tc.tile_pool(name="ht", bufs=1) as ht_p, \
         tc.tile_pool(name="acc", bufs=1) as acc_p, \
         tc.tile_pool(name="fin", bufs=4) as fin_p:
        env["acc_p"] = acc_p
        ht = {}

        def fin1(t, acc_t):
            tmp = fin_p.tile([128, 128], f32, tag="tmp")
            nc.vector.tensor_tensor(out=tmp[:], in0=acc_t[:],
                                    in1=drep_sb[:, t * 128:(t + 1) * 128], op=OP.mult)
            h_t = ht_p.tile([128, 128], dt_tab, tag=f"ht{t}")
            nc.scalar.activation(h_t[:], tmp[:], AF.Relu, bias=b1_sb[:])
            ht[t] = h_t

        if cfg.get("stop_after") == "G":
            propagate([g1_d[0][:, :], g1_d[1][:, :]], fin1, gathers_only=True)
            return
        with nc.named_scope("prop1"):
            propagate([g1_d[0][:, :], g1_d[1][:, :]], fin1)
        if cfg.get("stop_after") == "P1":
            return

        # ---------------- phase C: g2 shard = d * (h @ Wcat.T), AllGather
        sc2 = nc.named_scope("phaseC")
        sc2.__enter__()
        with tc.tile_pool(name="pc_t", bufs=4) as ct_p, \
             tc.tile_pool(name="pc_ps", bufs=2, space="PSUM") as pc1, \
             tc.tile_pool(name="pc_ps2", bufs=2, space="PSUM") as pc2:
            for t in range(T):
                ps = pc1.tile([O2, 128], f32, space="PSUM", tag="c1")
                nc.tensor.matmul(ps[:], lhsT=wcatt_sb[:], rhs=ht[t][:], start=True, stop=True)
                c_sb = ct_p.tile([O2, 128], dt_tab, tag="csb")
                nc.scalar.copy(c_sb[:], ps[:])
                ps2 = pc2.tile([128, O2], dt_tab, space="PSUM", tag="c2")
                nc.tensor.transpose(ps2[:], c_sb[:], ident_sb[:])
                g2t = ct_p.tile([128, O2], dt_tab, tag="g2t")
                nc.vector.tensor_scalar_mul(g2t[:], ps2[:], downc_sb[:, t:t + 1])
                nrows = min(128, NPC - t * 128)
                nc.sync.dma_start(g2s_d[t * 128:t * 128 + nrows, :], g2t[:nrows, :])
            nc.gpsimd.collective_compute(
                "AllGather",
                mybir.AluOpType.bypass,
                replica_groups=[list(range(C))],
                ins=[g2s_d[:]],
                outs=[g2f_d[:]],
            )
            nc.sync.dma_start(g2c_d[0][:, :], g2f_d[0:HS, :])
            nc.sync.dma_start(g2c_d[1][:, :], g2f_d[HS:N, :])

        sc2.__exit__(None, None, None)
        if cfg.get("stop_after") == "C":
            return

        # ---------------- phase D: second propagate + output
        def fin2(t, acc_t):
            tmp = fin_p.tile([128, 128], f32, tag="tmp")
            nc.vector.tensor_tensor(out=tmp[:], in0=acc_t[:],
                                    in1=drep_sb[:, t * 128:(t + 1) * 128], op=OP.mult)
            osb = fin_p.tile([O2, 128], f32, tag="osb")
            nc.scalar.activation(osb[:], tmp[:], AF.Identity, bias=bcat_sb[:])
            nc.sync.dma_start(outt_d[:, t * 128:(t + 1) * 128], osb[:])

        if cfg.get("stop_after") == "Dg1":
            propagate([g1_d[0][:, :], g1_d[1][:, :]], fin2)
        else:
            with nc.named_scope("prop2"):
                propagate([g2c_d[0][:, :], g2c_d[1][:, :]], fin2)


LAST_RESULTS = None


def run(cfg, x, edge_index, W1, b1, W_mu, b_mu, W_logstd, b_logstd, program_cache=None,
        trace=False, trace_cores=None):
    global LAST_RESULTS
    meta, in_maps = preprocess(cfg, x, edge_index, W1, b1, W_mu, b_mu, W_logstd, b_logstd)
    nc = build_program(cfg, meta)
    res = run_bass_kernel_spmd(nc, in_maps, list(range(cfg["n_cores"])),
                               trace=trace, trace_cores=trace_cores)
    LAST_RESULTS = res
    N, C = cfg["n"], cfg["n_cores"]
    NPC = N // C
    O = cfg["out2"] // 2
    mu = np.empty((N, O), np.float32)
    logstd = np.empty((N, O), np.float32)
    for c in range(C):
        ot = res.results[c]["outt"]
        mu[c * NPC:(c + 1) * NPC] = ot[:O, :NPC].T
        logstd[c * NPC:(c + 1) * NPC] = ot[O:, :NPC].T
    return mu, logstd


def kernel(x, edge_index, W1, b1, W_mu, b_mu, W_logstd, b_logstd):
    mu, logstd = run(FULL_CFG, x, edge_index, W1, b1, W_mu, b_mu, W_logstd, b_logstd)
    return mu, logstd



# revision 20
# speedup vs baseline: 2.7841x; 2.7841x over previous
"""Trainium2 Bass kernel for a 2-layer GCN encoder (GCNConv -> ReLU -> {GCNConv mu, GCNConv logstd}).

Strategy (8 NeuronCores, SPMD):
  - Math: propagate(M) = D^-1/2 (A+I) D^-1/2 M  ==  d * ((A+I) @ (d * M)) with d = deg^-1/2,
    so per-edge norm weights disappear: scale rows by d before and after message passing.
  - Layers 2 and 3 share the propagate: fuse W_mu/W_logstd into one [128,128] matmul + one
    message-passing pass over 128 features, split on the host afterwards.
  - Sharding: nodes are partitioned across the 8 cores (dst-sharding). Each core owns
    N/8 = 6250 output rows and processes the ~E/8 edges pointing into them.
  - Layer-1 linear (x @ W1.T) is replicated on every core (reads all of x, ~51 MB bf16)
    which avoids an all-gather of the layer-1 message table. Layer-2 tables require one
    AllGather of the per-core [6250,128] shards.
  - Message passing: dma_gather (HW gather, int16 indices) pulls source-node rows from the
    HBM-resident message table; a one-hot selection matrix (built on DVE via is_equal vs an
    iota row) turns the segment-sum into PE matmuls accumulated in PSUM per 128-dst-node tile.
  - int16 gather indices only address <=32767 rows, so node tables are split in two halves
    and each core's edge list is bucketed by source half (then by dst tile, padded to
    multiples of 128 with dummy edges whose one-hot column is out of range).
  - All cores run the same program (SPMD): per-(half,tile) group sizes are padded to the
    max over cores.

kernel(**inputs) takes the full-size inputs and returns (mu, logstd) as float32 numpy arrays.
"""
import sys

sys.path.insert(0, "/opt/trn_rl_repo")

import numpy as np
import ml_dtypes

import concourse.bass as bass
import concourse.bacc as bacc
import concourse.mybir as mybir
import concourse.tile as tile
from concourse.bass_utils import run_bass_kernel_spmd

BF16 = ml_dtypes.bfloat16

# ---------------- configuration ----------------
FULL_CFG = dict(
    n=50000,        # nodes
    fin=512,        # input features
    hid=128,        # hidden features
    out2=128,       # fused mu+logstd features
    n_cores=8,
    npad=50176,     # padded node count (8 * 6272; shard rows per core = 6272)
    hsplit=25088,   # node-table half split (both halves <= 32767)
    panel=2048,     # phase-A x_t panel width
    blk=512,        # phase-A block width
    g_edges=1024,   # gather super-chunk (edges per dma_gather)
    single_packet=True,  # <=1024 idxs: fire-and-forget, DMA drains async
    swdge_queues=4,
    gather_bufs=8,
    table_bf16=True,
    mm1_bf16=True,
    reps=1,         # kernel body repetitions (for timing)
)


def _ceil(a, b):
    return -(-a // b)


def preprocess(cfg, x, edge_index, W1, b1, W_mu, b_mu, W_logstd, b_logstd):
    """Host-side: degrees, edge bucketing/padding, operand staging. Returns
    (meta, in_maps). Pure index/layout work plus parameter reformatting."""
    N, C = cfg["n"], cfg["n_cores"]
    NPC = N // C
    T = _ceil(NPC, 128)
    HS = cfg["hsplit"]
    G = cfg["g_edges"]
    t_dt = BF16 if cfg["table_bf16"] else np.float32
    m_dt = BF16 if cfg["mm1_bf16"] else np.float32

    x = np.asarray(x, np.float32)
    ei = np.asarray(edge_index).astype(np.int64)
    W1 = np.asarray(W1, np.float32)
    b1 = np.asarray(b1, np.float32)
    Wcat = np.concatenate([np.asarray(W_mu, np.float32), np.asarray(W_logstd, np.float32)], axis=0)
    bcat = np.concatenate([np.asarray(b_mu, np.float32), np.asarray(b_logstd, np.float32)], axis=0)

    src = np.concatenate([ei[0], np.arange(N, dtype=np.int64)])
    dst = np.concatenate([ei[1], np.arange(N, dtype=np.int64)])
    deg = np.bincount(dst, minlength=N).astype(np.float32)
    dvec = (1.0 / np.sqrt(deg)).astype(np.float32)

    core = dst // NPC
    tloc = (dst % NPC) // 128
    half = (src >= HS).astype(np.int64)
    key = (core * T + tloc) * 2 + half
    order = np.argsort(key, kind="stable")
    ks, ss, ds = key[order], src[order], dst[order]
    counts = np.bincount(ks, minlength=C * T * 2).reshape(C, T, 2)
    gpad = ((counts.max(axis=0) + 127) // 128) * 128  # [T, 2] padded group sizes
    Lh = gpad.sum(axis=0)  # per-half padded edge totals (same for all cores)
    cpt = (gpad // 128)    # chunks per (tile, half)
    offs = np.concatenate([[0], np.cumsum(counts.reshape(-1))])

    # phase-A staging (sharded: core c computes table rows [c*SH, (c+1)*SH))
    NPAD = cfg["npad"]
    SH = NPAD // C
    xt = np.zeros((cfg["fin"], NPAD), m_dt)
    xt[:, :N] = x.T
    w1t = np.ascontiguousarray(W1.T).astype(m_dt)          # [fin, hid]
    wcatt = np.ascontiguousarray(Wcat.T).astype(t_dt)      # [hid, out2]
    d_all = np.ones(NPAD, np.float32)
    d_all[:N] = dvec
    iota_arr = np.tile(np.arange(128), (128, 1)).astype(t_dt)
    ident = np.eye(128, dtype=t_dt)
    identf = np.eye(128, dtype=np.float32)

    K_tot = int(Lh.sum() // 128)
    in_maps = []
    for c in range(C):
        xt_c = np.ascontiguousarray(xt[:, c * SH:(c + 1) * SH])
        dallc_c = np.ascontiguousarray(
            d_all[c * SH:(c + 1) * SH].reshape(-1, 128).T)  # [128, SH/128]
        # edge order: (tile, half) groups; gather idx streams are per half in
        # tile order; dstloc columns are (tile: half0-chunks then half1-chunks).
        bufs_ = [np.zeros(int(Lh[0]), np.int16), np.zeros(int(Lh[1]), np.int16)]
        ph = [0, 0]
        dstloc_all = np.full(int(Lh.sum()), 200, np.int32)
        pos = 0
        for t in range(T):
            for h in (0, 1):
                g = int(counts[c, t, h])
                o = int(offs[(c * T + t) * 2 + h])
                sl = slice(o, o + g)
                bufs_[h][ph[h]:ph[h] + g] = (ss[sl] - h * HS).astype(np.int16)
                dstloc_all[pos:pos + g] = (ds[sl] % NPC) - t * 128
                ph[h] += int(gpad[t, h])
                pos += int(gpad[t, h])
        idx_h = [np.tile(b.reshape(-1, 16).T, (8, 1)).copy() if b.size
                 else np.zeros((128, 1), np.int16) for b in bufs_]
        dstloc_arr = np.ascontiguousarray(dstloc_all.reshape(-1, 128).T).astype(t_dt)

        d_own = dvec[c * NPC:(c + 1) * NPC]
        d_own_pad = np.ones(T * 128, np.float32)
        d_own_pad[:NPC] = d_own
        d_rep = np.tile(d_own_pad, (128, 1)).astype(np.float32)          # [128, T*128]
        d_own_col = np.ascontiguousarray(d_own_pad.reshape(-1, 128).T)   # [128, T]

        in_maps.append({
            "xt": xt_c, "w1t": w1t, "wcatt": wcatt,
            "b1c": b1.reshape(-1, 1).copy(), "bcatc": bcat.reshape(-1, 1).copy(),
            "dallc": dallc_c, "drep": d_rep, "downc": d_own_col,
            "iota": iota_arr, "ident": ident, "identf": identf,
            "idx0": idx_h[0], "idx1": idx_h[1], "dstloc": dstloc_arr,
        })

    meta = dict(cpt=cpt.tolist(), Lh=[int(Lh[0]), int(Lh[1])], K_tot=K_tot)
    return meta, in_maps


def build_program(cfg, meta):
    N, C = cfg["n"], cfg["n_cores"]
    NPC = N // C
    T = _ceil(NPC, 128)
    HS, NPAD = cfg["hsplit"], cfg["npad"]
    FIN, HID, O2 = cfg["fin"], cfg["hid"], cfg["out2"]
    PANEL, BLK, G = cfg["panel"], cfg["blk"], cfg["g_edges"]
    KC = FIN // 128
    SPC = G // 128  # chunk slots per gather super-chunk
    cpt, Lh = meta["cpt"], meta["Lh"]
    K_tot = meta["K_tot"]
    dt_tab = mybir.dt.bfloat16 if cfg["table_bf16"] else mybir.dt.float32
    dt_mm = mybir.dt.bfloat16 if cfg["mm1_bf16"] else mybir.dt.float32
    f32 = mybir.dt.float32
    AF = mybir.ActivationFunctionType
    OP = mybir.AluOpType

    nc = bacc.Bacc("TRN2", target_bir_lowering=False, debug=False, num_devices=C,
                   num_swdge_queues=cfg.get("swdge_queues", 1))

    SH = NPAD // C
    xt_d = nc.dram_tensor("xt", [FIN, SH], dt_mm, kind="ExternalInput")
    w1t_d = nc.dram_tensor("w1t", [FIN, HID], dt_mm, kind="ExternalInput")
    wcatt_d = nc.dram_tensor("wcatt", [HID, O2], dt_tab, kind="ExternalInput")
    b1c_d = nc.dram_tensor("b1c", [HID, 1], f32, kind="ExternalInput")
    bcatc_d = nc.dram_tensor("bcatc", [O2, 1], f32, kind="ExternalInput")
    dallc_d = nc.dram_tensor("dallc", [128, SH // 128], f32, kind="ExternalInput")
    drep_d = nc.dram_tensor("drep", [128, T * 128], f32, kind="ExternalInput")
    downc_d = nc.dram_tensor("downc", [128, T], f32, kind="ExternalInput")
    iota_d = nc.dram_tensor("iota", [128, 128], dt_tab, kind="ExternalInput")
    ident_d = nc.dram_tensor("ident", [128, 128], dt_tab, kind="ExternalInput")
    identf_d = nc.dram_tensor("identf", [128, 128], f32, kind="ExternalInput")
    idx_d = [nc.dram_tensor(f"idx{h}", [128, max(Lh[h] // 16, 1)], mybir.dt.int16,
                            kind="ExternalInput") for h in (0, 1)]
    dstloc_d = nc.dram_tensor("dstloc", [128, max(K_tot, 1)], dt_tab, kind="ExternalInput")

    g1s_d = nc.dram_tensor("g1s", [SH, HID], dt_tab)
    g1f_d = nc.dram_tensor("g1f", [NPAD, HID], dt_tab, addr_space="Shared")
    g1c_d = [nc.dram_tensor("g1ca", [HS, HID], dt_tab),
             nc.dram_tensor("g1cb", [NPAD - HS, HID], dt_tab)]
    g2s_d = nc.dram_tensor("g2s", [NPC, HID], dt_tab)
    g2f_d = nc.dram_tensor("g2f", [N, HID], dt_tab, addr_space="Shared")
    # dma_gather cannot source from a Shared-address tensor (device crash);
    # bounce the collective output into plain DRAM halves.
    g2c_d = [nc.dram_tensor("g2ca", [HS, HID], dt_tab),
             nc.dram_tensor("g2cb", [N - HS, HID], dt_tab)]
    outt_d = nc.dram_tensor("outt", [O2, T * 128], f32, kind="ExternalOutput")

    with tile.TileContext(nc, trace_sim=bool(cfg.get("trace_sim"))) as tc:
        with tc.tile_pool(name="const", bufs=1) as const_p:
            w1t_sb = []
            for kc in range(KC):
                w = const_p.tile([128, HID], dt_mm, tag=f"w1t{kc}")
                nc.sync.dma_start(w[:], w1t_d[kc * 128:(kc + 1) * 128, :])
                w1t_sb.append(w)
            wcatt_sb = const_p.tile([HID, O2], dt_tab, tag="wcatt")
            nc.sync.dma_start(wcatt_sb[:], wcatt_d[:])
            b1_sb = const_p.tile([HID, 1], f32, tag="b1")
            nc.sync.dma_start(b1_sb[:], b1c_d[:])
            bcat_sb = const_p.tile([O2, 1], f32, tag="bcat")
            nc.sync.dma_start(bcat_sb[:], bcatc_d[:])
            dallc_sb = const_p.tile([128, (NPAD // C) // 128], f32, tag="dallc")
            nc.sync.dma_start(dallc_sb[:], dallc_d[:])
            drep_sb = const_p.tile([128, T * 128], f32, tag="drep")
            nc.sync.dma_start(drep_sb[:], drep_d[:])
            downc_sb = const_p.tile([128, T], f32, tag="downc")
            nc.sync.dma_start(downc_sb[:], downc_d[:])
            iota_sb = const_p.tile([128, 128], dt_tab, tag="iota")
            nc.sync.dma_start(iota_sb[:], iota_d[:])
            ident_sb = const_p.tile([128, 128], dt_tab, tag="ident")
            nc.sync.dma_start(ident_sb[:], ident_d[:])
            ident_f32_sb = const_p.tile([128, 128], f32, tag="identf")
            nc.sync.dma_start(ident_f32_sb[:], identf_d[:])
            idx_sb = []
            for h in (0, 1):
                t_ = const_p.tile([128, max(Lh[h] // 16, 1)], mybir.dt.int16, tag=f"idx{h}")
                nc.sync.dma_start(t_[:], idx_d[h][:])
                idx_sb.append(t_)
            dstloc_sb = const_p.tile([128, max(K_tot, 1)], dt_tab, tag="dstloc")
            nc.sync.dma_start(dstloc_sb[:], dstloc_d[:])

            for _rep in range(cfg.get("reps", 1)):
                with nc.named_scope("body"):
                    _emit_body(nc, tc, cfg, meta, locals())

    nc.compile()
    return nc


def _emit_body(nc, tc, cfg, meta, env):
    """One full forward pass. `env` carries the SBUF constants + DRAM handles."""
    N, C = cfg["n"], cfg["n_cores"]
    NPC = N // C
    T = _ceil(NPC, 128)
    HS, NPAD = cfg["hsplit"], cfg["npad"]
    FIN, HID, O2 = cfg["fin"], cfg["hid"], cfg["out2"]
    PANEL, BLK, G = cfg["panel"], cfg["blk"], cfg["g_edges"]
    KC = FIN // 128
    SPC = G // 128
    cpt, Lh = meta["cpt"], meta["Lh"]
    dt_tab = mybir.dt.bfloat16 if cfg["table_bf16"] else mybir.dt.float32
    dt_mm = mybir.dt.bfloat16 if cfg["mm1_bf16"] else mybir.dt.float32
    f32 = mybir.dt.float32
    AF = mybir.ActivationFunctionType
    OP = mybir.AluOpType

    xt_d, dstloc_sb, idx_sb = env["xt_d"], env["dstloc_sb"], env["idx_sb"]
    g2s_d, g2f_d, outt_d = env["g2s_d"], env["g2f_d"], env["outt_d"]
    g1s_d, g1f_d, g1c_d = env["g1s_d"], env["g1f_d"], env["g1c_d"]
    g2c_d = env["g2c_d"]
    w1t_sb, wcatt_sb = env["w1t_sb"], env["wcatt_sb"]
    b1_sb, bcat_sb = env["b1_sb"], env["bcat_sb"]
    dallc_sb, drep_sb, downc_sb = env["dallc_sb"], env["drep_sb"], env["downc_sb"]
    iota_sb, ident_sb = env["iota_sb"], env["ident_sb"]
    ident_f32_sb = env["ident_f32_sb"]

    # ---------------- phase A: g1 shard = d * (x @ W1.T) for own rows, then AllGather
    sc = nc.named_scope("phaseA")
    sc.__enter__()
    SH = NPAD // C
    with tc.tile_pool(name="pa_x", bufs=3) as xt_p, \
         tc.tile_pool(name="pa_t", bufs=3) as t1_p, \
         tc.tile_pool(name="pa_w", bufs=3) as wst_p, \
         tc.tile_pool(name="pa_ps", bufs=2, space="PSUM") as pa, \
         tc.tile_pool(name="pa_ps2", bufs=2, space="PSUM") as pb:
        blks = [BLK] * (SH // BLK) + ([SH % BLK] if SH % BLK else [])
        off = 0
        for bsz in blks:
            xts = []
            for kc in range(KC):
                xk = xt_p.tile([128, BLK], dt_mm, tag=f"xt{kc}")
                nc.sync.dma_start(xk[:, :bsz], xt_d[kc * 128:(kc + 1) * 128,
                                                    off:off + bsz])
                xts.append(xk)
            ps_a = pa.tile([128, BLK], f32, space="PSUM", tag="psa")
            for kc in range(KC):
                nc.tensor.matmul(ps_a[:, :bsz], lhsT=w1t_sb[kc][:],
                                 rhs=xts[kc][:, :bsz],
                                 start=(kc == 0), stop=(kc == KC - 1))
            t1t = t1_p.tile([128, BLK], dt_tab, tag="t1t")
            nc.scalar.copy(t1t[:, :bsz], ps_a[:, :bsz])
            sb = bsz // 128
            wst = wst_p.tile([128, BLK // 128, HID], dt_tab, tag="wst")
            ps_b = pb.tile([128, BLK // 128, 128], dt_tab, space="PSUM", tag="psb")
            for s in range(sb):
                nc.tensor.transpose(ps_b[:, s, :], t1t[:, s * 128:(s + 1) * 128],
                                    ident_sb[:])
            # scale all sub-blocks by their per-node d in one DVE op:
            # wst[p, s, f] = ps_b[p, s, f] * d_shard[(off/128+s)*128 + p]
            nb0 = off // 128
            dsl = dallc_sb[:, nb0:nb0 + sb]
            in1 = bass.AP(dsl.tensor, dsl.offset,
                          [dsl.ap[0], [dsl.ap[1][0], sb], [0, 128]])
            nc.vector.tensor_tensor(out=wst[:, :sb, :], in0=ps_b[:, :sb, :],
                                    in1=in1, op=OP.mult)
            nc.sync.dma_start(
                g1s_d[off:off + bsz, :].rearrange("(s p) f -> p s f", p=128),
                wst[:, :sb, :])
            off += bsz
        nc.gpsimd.collective_compute(
            "AllGather",
            mybir.AluOpType.bypass,
            replica_groups=[list(range(C))],
            ins=[g1s_d[:]],
            outs=[g1f_d[:]],
        )
        nc.sync.dma_start(g1c_d[0][:, :], g1f_d[0:HS, :])
        nc.sync.dma_start(g1c_d[1][:, :], g1f_d[HS:NPAD, :])

    sc.__exit__(None, None, None)
    if cfg.get("stop_after") == "A":
        return

    # ---------------- message passing (used for both layers)
    def propagate(tables, finalize, gathers_only=False):
        with tc.tile_pool(name="mp_g", bufs=cfg.get("gather_bufs", 2)) as gath_p, \
             tc.tile_pool(name="mp_oh", bufs=3) as oh_p, \
             tc.tile_pool(name="mp_ps", bufs=4, space="PSUM") as psp:
            gh = {}
            nq = cfg.get("swdge_queues", 1)
            qctr = 0
            for h in (0, 1):
                for i in range(_ceil(Lh[h], G)):
                    n_i = min(G, Lh[h] - i * G)
                    gt = gath_p.tile([128, SPC, HID], dt_tab, tag=f"g{h}", name="gt")
                    nc.gpsimd.dma_gather(
                        out_ap=gt[:, :n_i // 128, :],
                        in_ap=tables[h],
                        idxs_ap=idx_sb[h][:, i * (G // 16):i * (G // 16) + n_i // 16],
                        num_idxs=n_i,
                        num_idxs_reg=n_i,
                        elem_size=HID,
                        single_packet=cfg.get("single_packet", True),
                        queue_num=qctr % nq,
                    )
                    qctr += 1
                    gh[(h, i)] = gt
            if gathers_only:
                return
            # Two passes (half 0 then half 1) so half-0 chunk consumption only
            # depends on the half-0 table; partials accumulate in SBUF.
            kh = [0, 0]
            acc = {}
            for h in (0, 1):
                for t in range(T):
                    # dstloc columns for tile t: [kg0(t) .. ] half0 then half1
                    kg = sum(cpt[tt][0] + cpt[tt][1] for tt in range(t))
                    if h == 1:
                        kg += cpt[t][0]
                    nch = cpt[t][h]
                    if nch == 0:
                        if h == 1:
                            a = acc.get(t)
                            if a is None:
                                a = env["acc_p"].tile([128, 128], f32,
                                                      tag=f"acc{t}", name="acc_t")
                                nc.vector.memset(a[:], 0.0)
                            finalize(t, a)
                        continue
                    oh = oh_p.tile([128, nch, 128], dt_tab, tag="oh", name="oh")
                    dsl = dstloc_sb[:, kg:kg + nch]
                    in0 = bass.AP(dsl.tensor, dsl.offset,
                                  [dsl.ap[0], [dsl.ap[1][0], nch], [0, 128]])
                    io = iota_sb[:]
                    in1 = bass.AP(io.tensor, io.offset, [io.ap[0], [0, nch], io.ap[1]])
                    nc.vector.tensor_tensor(out=oh[:], in0=in0, in1=in1, op=OP.is_equal)
                    ps_t = psp.tile([128, 128], f32, space="PSUM", tag="ps", name="ps_t")
                    for j in range(nch):
                        gt = gh[(h, kh[h] // SPC)]
                        nc.tensor.matmul(ps_t[:], lhsT=gt[:, kh[h] % SPC, :],
                                         rhs=oh[:, j, :],
                                         start=(j == 0), stop=(j == nch - 1))
                        kh[h] += 1
                    if h == 0:
                        a = env["acc_p"].tile([128, 128], f32, tag=f"acc{t}", name="acc_t")
                        nc.scalar.copy(a[:], ps_t[:])
                        acc[t] = a
                    else:
                        a = acc.get(t)
                        if a is not None:
                            nc.vector.tensor_tensor(out=a[:], in0=a[:], in1=ps_t[:],
                                                    op=OP.add)
                            finalize(t, a)
                        else:
                            finalize(t, ps_t)

    with tc.tile_pool(name="ht", bufs=1) as ht_p, \
         tc.tile_pool(name="acc", bufs=1) as acc_p, \
         tc.tile_pool(name="fin", bufs=4) as fin_p:
        env["acc_p"] = acc_p
        ht = {}

        def fin1(t, acc_t):
            tmp = fin_p.tile([128, 128], f32, tag="tmp")
            nc.vector.tensor_tensor(out=tmp[:], in0=acc_t[:],
                                    in1=drep_sb[:, t * 128:(t + 1) * 128], op=OP.mult)
            h_t = ht_p.tile([128, 128], dt_tab, tag=f"ht{t}")
            nc.scalar.activation(h_t[:], tmp[:], AF.Relu, bias=b1_sb[:])
            ht[t] = h_t

        if cfg.get("stop_after") == "G":
            propagate([g1c_d[0][:, :], g1c_d[1][:, :]], fin1, gathers_only=True)
            return
        with nc.named_scope("prop1"):
            propagate([g1c_d[0][:, :], g1c_d[1][:, :]], fin1)
        if cfg.get("stop_after") == "P1":
            return

        # ---------------- phase C: g2 shard = d * (h @ Wcat.T), AllGather
        sc2 = nc.named_scope("phaseC")
        sc2.__enter__()
        with tc.tile_pool(name="pc_t", bufs=4) as ct_p, \
             tc.tile_pool(name="pc_ps", bufs=2, space="PSUM") as pc1, \
             tc.tile_pool(name="pc_ps2", bufs=2, space="PSUM") as pc2:
            for t in range(T):
                ps = pc1.tile([O2, 128], f32, space="PSUM", tag="c1")
                nc.tensor.matmul(ps[:], lhsT=wcatt_sb[:], rhs=ht[t][:], start=True, stop=True)
                c_sb = ct_p.tile([O2, 128], dt_tab, tag="csb")
                nc.scalar.copy(c_sb[:], ps[:])
                ps2 = pc2.tile([128, O2], dt_tab, space="PSUM", tag="c2")
                nc.tensor.transpose(ps2[:], c_sb[:], ident_sb[:])
                g2t = ct_p.tile([128, O2], dt_tab, tag="g2t")
                nc.vector.tensor_scalar_mul(g2t[:], ps2[:], downc_sb[:, t:t + 1])
                nrows = min(128, NPC - t * 128)
                nc.sync.dma_start(g2s_d[t * 128:t * 128 + nrows, :], g2t[:nrows, :])
            nc.gpsimd.collective_compute(
                "AllGather",
                mybir.AluOpType.bypass,
                replica_groups=[list(range(C))],
                ins=[g2s_d[:]],
                outs=[g2f_d[:]],
            )
            nc.sync.dma_start(g2c_d[0][:, :], g2f_d[0:HS, :])
            nc.sync.dma_start(g2c_d[1][:, :], g2f_d[HS:N, :])

        sc2.__exit__(None, None, None)
        if cfg.get("stop_after") == "C":
            return

        # ---------------- phase D: second propagate + output
        def fin2(t, acc_t):
            tmp = fin_p.tile([128, 128], f32, tag="tmp")
            nc.vector.tensor_tensor(out=tmp[:], in0=acc_t[:],
                                    in1=drep_sb[:, t * 128:(t + 1) * 128], op=OP.mult)
            osb = fin_p.tile([O2, 128], f32, tag="osb")
            nc.scalar.activation(osb[:], tmp[:], AF.Identity, bias=bcat_sb[:])
            nc.sync.dma_start(outt_d[:, t * 128:(t + 1) * 128], osb[:])

        if cfg.get("stop_after") == "Dg1":
            propagate([g1c_d[0][:, :], g1c_d[1][:, :]], fin2)
        else:
            with nc.named_scope("prop2"):
                propagate([g2c_d[0][:, :], g2c_d[1][:, :]], fin2)


LAST_RESULTS = None


def run(cfg, x, edge_index, W1, b1, W_mu, b_mu, W_logstd, b_logstd, program_cache=None,
        trace=False, trace_cores=None):
    global LAST_RESULTS
    meta, in_maps = preprocess(cfg, x, edge_index, W1, b1, W_mu, b_mu, W_logstd, b_logstd)
    nc = build_program(cfg, meta)
    res = run_bass_kernel_spmd(nc, in_maps, list(range(cfg["n_cores"])),
                               trace=trace, trace_cores=trace_cores)
    LAST_RESULTS = res
    N, C = cfg["n"], cfg["n_cores"]
    NPC = N // C
    O = cfg["out2"] // 2
    mu = np.empty((N, O), np.float32)
    logstd = np.empty((N, O), np.float32)
    for c in range(C):
        ot = res.results[c]["outt"]
        mu[c * NPC:(c + 1) * NPC] = ot[:O, :NPC].T
        logstd[c * NPC:(c + 1) * NPC] = ot[O:, :NPC].T
    return mu, logstd


def kernel(x, edge_index, W1, b1, W_mu, b_mu, W_logstd, b_logstd):
    mu, logstd = run(FULL_CFG, x, edge_index, W1, b1, W_mu, b_mu, W_logstd, b_logstd)
    return mu, logstd

